# revision 26
# baseline (speedup 1.0000x reference)
"""ChildSumTreeLSTMCell on 8 Trainium2 NeuronCores.

Strategy: sort edges by destination node on the host, partition nodes
contiguously across the 8 cores so every core's segment sums are fully
local -- zero collectives.  Edges are packed into subtiles (<=64 nodes,
<=256 edges = 2 chunks of 128 slots); 8 subtiles form a superblock
(512 node slots, 16 chunks, 2048 edge slots).  Segment sums are matmuls
against a 0/1 membership matrix built on the HOST and shipped as fp8.

v4 plan: h / embed / sf / df shipped as float8e3 (e3m4: 2x the mantissa
of e4m3, so embed can drop from bf16 to 1 byte at equal accuracy),
c stays bf16, membership fp8e4.  hw4 = h*ew requantized to fp8e4 so the
h-part segment sum runs as one DoubleRow matmul per subtile (256-deep
fp8 contraction).  cs stays in PSUM until the ct = f*cs multiply (no
evacuation); only hs (h|e halves) is evacuated by ACT, ahead of the
gate sigmoids in the ACT queue.  Weight/bias constants ship as ONE
batched f32 DMA and the first input pair is fetched before anything
else so compute starts ~12us in.
"""

import sys

for _p in ("/opt/trn_rl_repo", "/root/.axon_site/_ro/trn_rl_repo"):
    if _p not in sys.path:
        sys.path.append(_p)

import numpy as np
import ml_dtypes

import concourse.bacc as bacc
import concourse.mybir as mybir
import concourse.tile as tile
from concourse.bass_utils import run_bass_kernel_spmd

F32 = mybir.dt.float32
BF16 = mybir.dt.bfloat16
F8E4 = mybir.dt.float8e4
F8E3 = mybir.dt.float8e3

E = 500_000
N = 125_000
H = 128
G = 64
NCORES = 8
NPC = N // NCORES          # nodes per core
CHUNK = 128                # edge slots per chunk (contraction width)
SUB_N = 64                 # node slots per subtile
SUB_C = 2                  # chunks per subtile
SUB_E = SUB_C * CHUNK      # edge slots per subtile
SPB = 8                    # subtiles per superblock
BLK_C = SPB * SUB_C        # chunks per superblock (16)
BLK_E = SPB * SUB_E        # edge slots per superblock (2048)
BLK_N = SPB * SUB_N        # node slots per superblock (512)

bf16_np = ml_dtypes.bfloat16
f8e4_np = ml_dtypes.float8_e4m3
f8e3_np = ml_dtypes.float8_e3m4

TRACE = False              # set by test.py to capture an NTFF profile
LAST = {}                  # last run's BassKernelResults

# wall layout (single [128, WALL_W] f32 constant DMA)
WG_OFF = 0                 # 8 x [128,128]: (Wf a|b, Wi a|b, Wu a|b, Wo a|b)^T
WEL_OFF = 1024             # [64, 128] W_el^T on partitions 0:64
WA_OFF = 1152              # [64, 4]  [W_eoh | b_eoh] on partitions 0:64
BEL_OFF = 1156             # [4, 128] row 3 = b_el, on partitions 0:4
BIAS_OFF = 1284            # 8 x [128,1]: bWf, bf, bWi, bi, bWu, bu, bWo, bo
WALL_W = 1292


def _install_axon_hook():
    import types, contextlib, ctypes

    def _make_hook(so_path="/opt/axon/libaxon_pjrt.so"):
        lib = ctypes.CDLL(so_path)
        if not hasattr(lib, "axon_start_nrt_profile"):
            return None
        lib.axon_start_nrt_profile.argtypes = [
            ctypes.POINTER(ctypes.c_int64), ctypes.c_size_t]
        lib.axon_start_nrt_profile.restype = ctypes.c_int64
        lib.axon_stop_nrt_profile.argtypes = [ctypes.c_char_p]
        lib.axon_stop_nrt_profile.restype = ctypes.c_int64

        @contextlib.contextmanager
        def hook(output_dir, device_ids):
            import jax
            jax.devices()
            if device_ids:
                ids = (ctypes.c_int64 * len(device_ids))(*device_ids)
                rc = lib.axon_start_nrt_profile(ids, len(device_ids))
            else:
                rc = lib.axon_start_nrt_profile(None, 0)
            if rc != 0:
                raise RuntimeError("axon_start_nrt_profile rc=%d" % rc)
            try:
                yield
            finally:
                n = lib.axon_stop_nrt_profile(str(output_dir).encode())
                print("profile: %d file(s) written to %s" % (n, output_dir),
                      file=sys.stderr)

        return hook

    hook = _make_hook()
    mod = types.ModuleType("antenv.axon_hooks")
    mod.get_axon_ntff_profile_hook = lambda: hook
    mod.set_axon_ntff_profile_hook = lambda h: None
    sys.modules["antenv.axon_hooks"] = mod


def build_graph(SB):
    """Per-core Bass graph for SB superblocks."""
    nc = bacc.Bacc()
    assert SB % 2 == 0 and SB >= 6
    dp = nc.declare_dram_parameter
    D8W = 2 * BLK_E            # per-sb cols in d8: h 2048 | e 2048
    d8_ext = dp("d8", [SB // 2, 128, 2 * D8W], F8E3, isOutput=False)
    dm4_ext = dp("dm4", [SB // 2, 128, 2 * BLK_N * SUB_C], F8E4, isOutput=False)
    d16_ext = dp("d16", [SB // 2, 128, 2 * BLK_E], BF16, isOutput=False)
    SFW = 2 * BLK_E            # per-sb cols in sfdf: sf 2048 | df 2048
    sfdf_ext = dp("sfdf", [SB // 2, G + 4, 2 * SFW], F8E3, isOutput=False)
    wall_ext = dp("wall", [128, WALL_W], F32, isOutput=False)
    out_ext = dp("outT", [128, SB * 2 * BLK_N], BF16, isOutput=True)

    AF = mybir.ActivationFunctionType
    PM = mybir.MatmulPerfMode
    M4W = BLK_N * SUB_C        # per-sb cols in dm4 (1024)

    with tile.TileContext(nc) as tc:
        cst = tc.alloc_tile_pool(name="cst", bufs=1)
        pin = tc.alloc_tile_pool(name="pin", bufs=5)
        pcv = tc.alloc_tile_pool(name="pcv", bufs=3)
        pnd = tc.alloc_tile_pool(name="pnd", bufs=3)
        pew = tc.alloc_tile_pool(name="pew", bufs=2, space="PSUM")
        phs = tc.alloc_tile_pool(name="phs", bufs=1, space="PSUM")
        pcs = tc.alloc_tile_pool(name="pcs", bufs=2, space="PSUM")
        pgp = tc.alloc_tile_pool(name="pgp", bufs=2, space="PSUM")

        # -- constants first: small DMA on the scalar hwdge queue, which
        # carries no bulk traffic, so wtcomb assembly finishes ~4us in ------
        wall = cst.tile([128, WALL_W], F32)
        nc.scalar.dma_start(out=wall[:], in_=wall_ext[:])

        # Input fetches split over two DMA queue groups: d8+m4 on the sync
        # (SP hwdge) queue, sfdf+d16 on the gpsimd queue.  The scalar queue
        # only carries constants — in-loop triggers on a busy engine queue
        # block it when they wait for a free tile buffer.  sfdf first
        # within the pair (unblocks B68 + ew).
        def fetch_pair(p):
            sfdf = pin.tile([G + 4, 2 * SFW], F8E3, tag="sfdf")
            nc.gpsimd.dma_start(out=sfdf[:], in_=sfdf_ext[p])
            d8 = pin.tile([128, 2 * D8W], F8E3, tag="d8")
            nc.sync.dma_start(out=d8[:], in_=d8_ext[p])
            m4 = pin.tile([128, 2 * M4W], F8E4, tag="m4")
            nc.sync.dma_start(out=m4[:], in_=dm4_ext[p])
            d16 = pin.tile([128, 2 * BLK_E], BF16, tag="d16")
            nc.gpsimd.dma_start(out=d16[:], in_=d16_ext[p])
            return (d8, d16, m4, sfdf)

        pairs = {0: fetch_pair(0), 1: fetch_pair(1), 2: fetch_pair(2)}

        t2p = pgp.tile([4, 128], F32, tag="mm")
        nc.tensor.matmul(out=t2p[:], lhsT=wall[0:G, WA_OFF:WA_OFF + 4],
                         rhs=wall[0:G, WEL_OFF:WEL_OFF + 128],
                         start=True, stop=True)
        t4b = cst.tile([4, 128], BF16)
        nc.vector.tensor_tensor(out=t4b[:], in0=t2p[:],
                                in1=wall[0:4, BEL_OFF:BEL_OFF + 128],
                                op=mybir.AluOpType.add)
        wel_b16 = cst.tile([G, 128], BF16)
        nc.vector.tensor_copy(out=wel_b16[:],
                              in_=wall[0:G, WEL_OFF:WEL_OFF + 128])
        wtcomb = cst.tile([G + 4, 128], BF16)
        nc.scalar.dma_start(out=wtcomb[0:G, :], in_=wel_b16[:])
        nc.scalar.dma_start(out=wtcomb[G:G + 4, :], in_=t4b[:])

        wg = {}
        for xi, x in enumerate("fiuo"):
            wa_t = cst.tile([128, 128], BF16, tag="wg_%s_a" % x)
            nc.vector.tensor_copy(
                out=wa_t[:], in_=wall[:, 256 * xi:256 * xi + 128])
            wb_t = cst.tile([128, 128], BF16, tag="wg_%s_b" % x)
            nc.gpsimd.tensor_copy(
                out=wb_t[:], in_=wall[:, 256 * xi + 128:256 * xi + 256])
            wg[x] = (wa_t, wb_t)

        bias = {}
        for xi, x in enumerate("fiuo"):
            bs = cst.tile([128, 1], F32, tag="bs_%s" % x)
            nc.vector.tensor_tensor(
                out=bs[:], in0=wall[:, BIAS_OFF + 2 * xi:BIAS_OFF + 2 * xi + 1],
                in1=wall[:, BIAS_OFF + 2 * xi + 1:BIAS_OFF + 2 * xi + 2],
                op=mybir.AluOpType.add)
            bias[x] = bs

        # -- per-superblock emission helpers --------------------------------
        # B68 = sf*df products; Pool is ~2.1x slower per column than DVE so
        # the split is asymmetric, and the two halves are emitted at
        # different points (Pool early, DVE late) to order each queue well.
        BQ = 704                   # B68 columns computed on DVE

        def emit_b68_pool(g, sfdf):
            j = (g % 2) * SFW
            B68 = pcv.tile([G + 4, BLK_E], F8E4, tag="B68")
            nc.gpsimd.tensor_tensor(
                out=B68[:, BQ:BLK_E],
                in0=sfdf[:, j + BQ:j + BLK_E],
                in1=sfdf[:, j + BLK_E + BQ:j + 2 * BLK_E],
                op=mybir.AluOpType.mult)
            return B68

        def emit_b68_dve(g, B68, sfdf):
            j = (g % 2) * SFW
            nc.vector.tensor_tensor(
                out=B68[:, 0:BQ], in0=sfdf[:, j:j + BQ],
                in1=sfdf[:, j + BLK_E:j + BLK_E + BQ],
                op=mybir.AluOpType.mult)

        def emit_gates(g, hsab):
            gate = {}
            for x, fn in (("f", "Sigmoid"), ("i", "Sigmoid"),
                          ("u", "Tanh"), ("o", "Sigmoid")):
                gp = pgp.tile([128, BLK_N], F32, tag="mm")
                nc.tensor.matmul(out=gp[:], lhsT=wg[x][0][:],
                                 rhs=hsab[:, 0:BLK_N], start=True, stop=False)
                nc.tensor.matmul(out=gp[:], lhsT=wg[x][1][:],
                                 rhs=hsab[:, BLK_N:2 * BLK_N],
                                 start=False, stop=True)
                gs = pnd.tile([128, BLK_N], BF16, tag="g_%s" % x)
                nc.scalar.activation(out=gs[:], in_=gp[:],
                                     func=getattr(AF, fn), bias=bias[x][:])
                gate[x] = gs
            return gate

        hc2_ref = [None]

        def emit_assembly_head(g, gate, cs_ps):
            # node-level, all on DVE (bf16 2x) + ACT tanh: ct = f*cs (cs read
            # straight from PSUM), c = i*u + ct, th = tanh(c)
            ct = pnd.tile([128, BLK_N], BF16, tag="ct")
            nc.vector.tensor_tensor(out=ct[:], in0=gate["f"][:],
                                    in1=cs_ps[:], op=mybir.AluOpType.mult)
            iu = pnd.tile([128, BLK_N], BF16, tag="iu")
            nc.vector.tensor_tensor(out=iu[:], in0=gate["i"][:],
                                    in1=gate["u"][:], op=mybir.AluOpType.mult)
            if g % 2 == 0:
                hc2_ref[0] = pnd.tile([128, 4 * BLK_N], BF16, tag="hc",
                                      name="hc2")
            hc = hc2_ref[0]
            o2 = (g % 2) * 2 * BLK_N
            nc.vector.tensor_tensor(out=hc[:, o2 + BLK_N:o2 + 2 * BLK_N],
                                    in0=iu[:], in1=ct[:],
                                    op=mybir.AluOpType.add)
            th = pnd.tile([128, BLK_N], BF16, tag="th")
            nc.scalar.activation(out=th[:], in_=hc[:, o2 + BLK_N:
                                                  o2 + 2 * BLK_N],
                                 func=AF.Tanh)
            return (g, gate, hc, th)

        def emit_assembly_tail(g, gate, hc, th):
            # h = o*tanh(c) on Pool (it idles at this point of the
            # pipeline; on DVE this op measured 3-5x slow — it runs right
            # when the DVE would rather stream the next hw4).  The output
            # DMA trigger rides the SAME engine queue directly behind hmul
            # so it never blocks the queue waiting on hc; output ships in
            # 2-superblock batches.
            o2 = (g % 2) * 2 * BLK_N
            nc.gpsimd.tensor_tensor(out=hc[:, o2:o2 + BLK_N],
                                    in0=gate["o"][:],
                                    in1=th[:], op=mybir.AluOpType.mult)
            if g % 2 == 1 or g == SB - 1:
                g0 = g - (g % 2)
                nc.gpsimd.dma_start(
                    out=out_ext[:, g0 * 2 * BLK_N:(g0 + 2) * 2 * BLK_N],
                    in_=hc[:])

        # -- main loop -------------------------------------------------------
        # Software pipeline: B68 products run TWO superblocks ahead and the
        # whole ew->hw4 stream runs ONE superblock ahead, so PE's h-segsum
        # never waits on same-iteration DVE work.
        def d8h_of(j):
            d8p = pairs[j // 2][0]
            return d8p[:, (j % 2) * D8W:(j % 2) * D8W + BLK_E]

        def ew_quarter(j, q, B68j, hw4j):
            # 4 ew matmuls -> one hw4 = h*ew quarter on DVE (superblock j)
            ew_ps = pew.tile([128, 512], F32, tag="ew")
            for c in range(4):
                ch = q * 4 + c
                nc.tensor.matmul(
                    out=ew_ps[:, c * 128:(c + 1) * 128],
                    lhsT=B68j[:, ch * 128:(ch + 1) * 128],
                    rhs=wtcomb[:], start=True, stop=True)
            nc.vector.tensor_tensor(
                out=hw4j[:, q * 512:(q + 1) * 512],
                in0=d8h_of(j)[:, q * 512:(q + 1) * 512], in1=ew_ps[:],
                op=mybir.AluOpType.mult)

        B68s = {0: emit_b68_pool(0, pairs[0][3])}
        emit_b68_dve(0, B68s[0], pairs[0][3])
        B68s[1] = emit_b68_pool(1, pairs[0][3])
        emit_b68_dve(1, B68s[1], pairs[0][3])
        hw4s = {0: pcv.tile([128, BLK_E], F8E4, tag="hw4", name="hw40")}
        for q in range(4):
            ew_quarter(0, q, B68s[0], hw4s[0])

        prev = None     # (g, gate, cs_ps) awaiting assembly
        evac = None     # (g, hsx, cs_ps) awaiting gates
        for g in range(SB):
            if g % 2 == 1 and (g + 5) // 2 < SB // 2:
                pairs[(g + 5) // 2] = fetch_pair((g + 5) // 2)
            d8p, d16p, m4p, _ = pairs[g // 2]
            j8 = (g % 2) * D8W
            jm = (g % 2) * M4W
            j16 = (g % 2) * BLK_E
            d8e = d8p[:, j8 + BLK_E:j8 + D8W]
            m4 = m4p[:, jm:jm + M4W]
            d16c = d16p[:, j16:j16 + BLK_E]

            if g + 2 < SB:
                B68s[g + 2] = emit_b68_pool(g + 2, pairs[(g + 2) // 2][3])
            B68nx = B68s.get(g + 1)
            hw4 = hw4s[g]
            if g + 1 < SB:
                hw4s[g + 1] = pcv.tile([128, BLK_E], F8E4, tag="hw4",
                                       name="hw4n")

            # node-level head of superblock g-1 (ct/iu/add lead the DVE
            # queue so the c-segsum + pnd consumers unblock early)
            asm = None
            if prev is not None:
                asm = emit_assembly_head(prev[0], prev[1], prev[2])
                prev = None

            hs = phs.tile([128, 2 * BLK_N], F32, tag="hs")
            cs = pcs.tile([128, BLK_N], F32, tag="cs")

            def cseg(st_lo, st_hi):
                for st in range(st_lo, st_hi):
                    for k in range(SUB_C):
                        ch = st * SUB_C + k
                        nc.tensor.matmul(
                            out=cs[:, st * SUB_N:(st + 1) * SUB_N],
                            lhsT=d16c[:, ch * 128:(ch + 1) * 128],
                            rhs=m4[:, ch * SUB_N:(ch + 1) * SUB_N],
                            start=(k == 0), stop=(k == SUB_C - 1))

            def eseg(st_lo, st_hi):
                for st in range(st_lo, st_hi):
                    for k in range(SUB_C):
                        ch = st * SUB_C + k
                        nc.tensor.matmul(
                            out=hs[:, BLK_N + st * SUB_N:
                                   BLK_N + (st + 1) * SUB_N],
                            lhsT=d8e[:, ch * 128:(ch + 1) * 128],
                            rhs=m4[:, ch * SUB_N:(ch + 1) * SUB_N],
                            start=(k == 0), stop=(k == SUB_C - 1))

            def hseg(st_lo, st_hi):
                # h-part segment sum: one DoubleRow fp8 matmul per subtile;
                # hw4 of THIS superblock was produced last iteration
                for st in range(st_lo, st_hi):
                    nc.tensor.matmul(
                        out=hs[:, st * SUB_N:(st + 1) * SUB_N],
                        lhsT=hw4[:, st * SUB_E:(st + 1) * SUB_E].rearrange(
                            "p (k m) -> p k m", k=2),
                        rhs=m4[:, st * SUB_C * SUB_N:(st + 1) * SUB_C * SUB_N]
                            .rearrange("p (k n) -> p k n", k=2),
                        start=True, stop=True, perf_mode=PM.DoubleRow)

            def phase(q):
                if g + 1 < SB:
                    ew_quarter(g + 1, q, B68nx, hw4s[g + 1])
                cseg(2 * q, 2 * q + 2)
                eseg(2 * q, 2 * q + 2)
                hseg(2 * q, 2 * q + 2)

            phase(0)
            phase(1)
            phase(2)

            # h = o*tanh(c) of g-1 (Pool) + its output DMA
            if asm is not None:
                emit_assembly_tail(*asm)
                asm = None

            # gates of the previous superblock between phases so the ACT
            # sigmoids land early while PE still has phase-3 work queued
            if evac is not None:
                prev = (evac[0], emit_gates(evac[0], evac[1]), evac[2])

            phase(3)

            # evacuate hs (ACT) behind the gate sigmoids
            hsx = pnd.tile([128, 2 * BLK_N], BF16, tag="hsx")
            nc.scalar.activation(out=hsx[:], in_=hs[:], func=AF.Copy)

            evac = (g, hsx, cs)
            if g + 2 < SB:
                emit_b68_dve(g + 2, B68s[g + 2], pairs[(g + 2) // 2][3])
            hw4s.pop(g, None)
            B68s.pop(g, None)

        if prev is not None:
            asm = emit_assembly_head(prev[0], prev[1], prev[2])
            emit_assembly_tail(*asm)
        prev = (evac[0], emit_gates(evac[0], evac[1]), evac[2])
        asm = emit_assembly_head(prev[0], prev[1], prev[2])
        emit_assembly_tail(*asm)

        for p in (pgp, pcs, phs, pew, pnd, pcv, pin, cst):
            p.release()
    nc.finalize()
    return nc


def plan_subtiles(dst_local, npc):
    """Greedy: <=SUB_N nodes and <=SUB_E edges per subtile.
    Returns list of (n0, n1, e0, e1) using sorted-edge offsets."""
    cnt = np.bincount(dst_local, minlength=npc)
    cum = np.concatenate([[0], np.cumsum(cnt)])
    tiles = []
    s = 0
    while s < npc:
        hi = min(s + SUB_N, npc)
        m = int(np.searchsorted(cum, cum[s] + SUB_E, side="right")) - 1
        m = max(s + 1, min(hi, m))
        tiles.append((s, m, int(cum[s]), int(cum[m])))
        s = m
    return tiles


def prep_core(k, h_src, c_src, embed_dst, src_f, dst_f, etype, dst, SB):
    """Build one core's padded superblock arrays."""
    lo = k * NPC
    sel = np.nonzero((dst >= lo) & (dst < lo + NPC))[0]
    dl = (dst[sel] - lo).astype(np.int64)
    order = np.argsort(dl, kind="stable")
    eidx = sel[order]
    dls = dl[order]
    tiles = plan_subtiles(dls, NPC)
    T = SB * SPB
    assert len(tiles) <= T
    ES = T * SUB_E
    src_slot = np.full(ES, -1, dtype=np.int64)
    nl_slot = np.zeros(ES, dtype=np.int64)      # node idx within subtile
    for t, (n0, n1, e0, e1) in enumerate(tiles):
        ne = e1 - e0
        assert ne <= SUB_E and n1 - n0 <= SUB_N
        src_slot[t * SUB_E:t * SUB_E + ne] = eidx[e0:e1]
        nl_slot[t * SUB_E:t * SUB_E + ne] = dls[e0:e1] - n0
    val = src_slot >= 0
    gi = src_slot[val]

    def pad_rows(a, w):
        out = np.zeros((ES, w), dtype=np.float32)
        out[val] = a[gi]
        return out

    def chunk_layout(a, w):
        # [ES, w] -> [SB, 128, BLK_C*w]: slot (sb, ch, p) dim d at
        # [sb, p, ch*w + d]
        return np.ascontiguousarray(
            a.reshape(SB, BLK_C, CHUNK, w).transpose(0, 2, 1, 3)
             .reshape(SB, 128, BLK_C * w))

    # membership: [sb, p, ch*64 + j] = (nl_slot of (sb,ch,p) == j)
    nl = nl_slot.reshape(SB, BLK_C, CHUNK)
    vl = val.reshape(SB, BLK_C, CHUNK)
    m4 = (nl[:, :, :, None] == np.arange(SUB_N)[None, None, None, :])
    m4 = (m4 & vl[:, :, :, None]).astype(np.float32)
    m4 = m4.reshape(SB, BLK_C, CHUNK, SUB_N).transpose(0, 2, 1, 3) \
           .reshape(SB, 128, BLK_C * SUB_N)

    def pair(a):
        # [SB, P, W] -> [SB/2, P, 2W]
        S, P, W = a.shape
        return np.ascontiguousarray(
            a.reshape(S // 2, 2, P, W).transpose(0, 2, 1, 3)
             .reshape(S // 2, P, 2 * W))

    h8 = chunk_layout(pad_rows(h_src, H), H)
    e8 = chunk_layout(pad_rows(embed_dst, H), H)
    d8 = pair(np.concatenate([h8, e8], axis=2)).astype(f8e3_np)
    dm4 = pair(m4).astype(f8e4_np)

    cp = chunk_layout(pad_rows(c_src, H), H)
    d16 = pair(cp).astype(bf16_np)

    # sf' = [sf | onehot4], df' = [df | ones]: [SB, 68, ch*128 + p]
    sfp = np.zeros((ES, G + 4), dtype=np.float32)
    sfp[val, :G] = src_f[gi]
    sfp[val, G + etype[gi]] = 1.0
    sfp[val, G + 3] = 1.0
    dfp = np.zeros((ES, G + 4), dtype=np.float32)
    dfp[val, :G] = dst_f[gi]
    dfp[val, G:] = 1.0
    def feat_layout(a):
        return a.reshape(SB, BLK_C * CHUNK, G + 4).transpose(0, 2, 1)
    sfdf = pair(np.concatenate(
        [feat_layout(sfp), feat_layout(dfp)], axis=2)).astype(f8e3_np)

    return {"d8": d8, "dm4": dm4, "d16": d16, "sfdf": sfdf}, tiles


def build_wall(inputs):
    wall = np.zeros((128, WALL_W), dtype=np.float32)
    for xi, (wn, bwn, bn) in enumerate(
            (("Wf", "bWf", "bf"), ("Wi", "bWi", "bi"),
             ("Wu", "bWu", "bu"), ("Wo", "bWo", "bo"))):
        wT = np.asarray(inputs[wn], np.float32).T         # [256, 128]
        wall[:, 256 * xi:256 * xi + 128] = wT[0:128]
        wall[:, 256 * xi + 128:256 * xi + 256] = wT[128:256]
        wall[:, BIAS_OFF + 2 * xi] = np.asarray(inputs[bwn], np.float32)
        wall[:, BIAS_OFF + 2 * xi + 1] = np.asarray(inputs[bn], np.float32)
    wall[0:G, WEL_OFF:WEL_OFF + 128] = np.asarray(inputs["W_el"], np.float32).T
    wall[0:G, WA_OFF:WA_OFF + 3] = np.asarray(inputs["W_eoh"], np.float32)
    wall[0:G, WA_OFF + 3] = np.asarray(inputs["b_eoh"], np.float32)
    wall[3, BEL_OFF:BEL_OFF + 128] = np.asarray(inputs["b_el"], np.float32)
    return wall


_graph_cache = {}


def kernel(**inputs):
    h_src = np.asarray(inputs["h_src"], dtype=np.float32)
    c_src = np.asarray(inputs["c_src"], dtype=np.float32)
    embed_dst = np.asarray(inputs["embed_dst"], dtype=np.float32)
    src_f = np.asarray(inputs["src_node_feat"], dtype=np.float32)
    dst_f = np.asarray(inputs["dst_node_feat"], dtype=np.float32)
    etype = np.asarray(inputs["edge_type_idx"]).astype(np.int64)
    dst = np.asarray(inputs["dst_idx"]).astype(np.int64)

    wall = build_wall(inputs)

    planned = []
    for k in range(NCORES):
        lo = k * NPC
        sel = np.nonzero((dst >= lo) & (dst < lo + NPC))[0]
        dl = np.sort((dst[sel] - lo).astype(np.int64))
        planned.append(plan_subtiles(dl, NPC))
    T = max(len(p) for p in planned)
    SB = (T + SPB - 1) // SPB
    SB += SB % 2
    SB = max(SB, 6)

    in_maps = []
    tiles_all = []
    for k in range(NCORES):
        m, tiles = prep_core(k, h_src, c_src, embed_dst, src_f, dst_f,
                             etype, dst, SB)
        m["wall"] = wall
        in_maps.append(m)
        tiles_all.append(tiles)

    if SB not in _graph_cache:
        _graph_cache[SB] = build_graph(SB)
    nc = _graph_cache[SB]

    if TRACE:
        _install_axon_hook()
    res = run_bass_kernel_spmd(nc, in_maps, list(range(NCORES)), trace=TRACE)
    LAST["res"] = res

    out = np.empty((N, 2 * H), dtype=np.float32)
    for k in range(NCORES):
        outT = np.asarray(res.results[k]["outT"]).astype(np.float32)
        for t, (n0, n1, _, _) in enumerate(tiles_all[k]):
            nn = n1 - n0
            base = k * NPC
            sb, st = divmod(t, SPB)
            col = sb * 2 * BLK_N + st * SUB_N
            out[base + n0:base + n1, 0:H] = outT[:, col:col + nn].T
            out[base + n0:base + n1, H:2 * H] = \
                outT[:, col + BLK_N:col + BLK_N + nn].T
    return out


# revision 28
# speedup vs baseline: 1.0044x; 1.0044x over previous
"""ChildSumTreeLSTMCell on 8 Trainium2 NeuronCores.

Strategy: sort edges by destination node on the host, partition nodes
contiguously across the 8 cores so every core's segment sums are fully
local -- zero collectives.  Edges are packed into subtiles (<=64 nodes,
<=256 edges = 2 chunks of 128 slots); 8 subtiles form a superblock
(512 node slots, 16 chunks, 2048 edge slots).  Segment sums are matmuls
against a 0/1 membership matrix built on the HOST and shipped as fp8.

v4 plan: h / embed / sf / df shipped as float8e3 (e3m4: 2x the mantissa
of e4m3, so embed can drop from bf16 to 1 byte at equal accuracy),
c stays bf16, membership fp8e4.  hw4 = h*ew requantized to fp8e4 so the
h-part segment sum runs as one DoubleRow matmul per subtile (256-deep
fp8 contraction).  cs stays in PSUM until the ct = f*cs multiply (no
evacuation); only hs (h|e halves) is evacuated by ACT, ahead of the
gate sigmoids in the ACT queue.  Weight/bias constants ship as ONE
batched f32 DMA and the first input pair is fetched before anything
else so compute starts ~12us in.
"""

import sys

for _p in ("/opt/trn_rl_repo", "/root/.axon_site/_ro/trn_rl_repo"):
    if _p not in sys.path:
        sys.path.append(_p)

import numpy as np
import ml_dtypes

import concourse.bacc as bacc
import concourse.mybir as mybir
import concourse.tile as tile
from concourse.bass_utils import run_bass_kernel_spmd

F32 = mybir.dt.float32
BF16 = mybir.dt.bfloat16
F8E4 = mybir.dt.float8e4
F8E3 = mybir.dt.float8e3

E = 500_000
N = 125_000
H = 128
G = 64
NCORES = 8
NPC = N // NCORES          # nodes per core
CHUNK = 128                # edge slots per chunk (contraction width)
SUB_N = 64                 # node slots per subtile
SUB_C = 2                  # chunks per subtile
SUB_E = SUB_C * CHUNK      # edge slots per subtile
SPB = 8                    # subtiles per superblock
BLK_C = SPB * SUB_C        # chunks per superblock (16)
BLK_E = SPB * SUB_E        # edge slots per superblock (2048)
BLK_N = SPB * SUB_N        # node slots per superblock (512)

bf16_np = ml_dtypes.bfloat16
f8e4_np = ml_dtypes.float8_e4m3
f8e3_np = ml_dtypes.float8_e3m4

TRACE = False              # set by test.py to capture an NTFF profile
LAST = {}                  # last run's BassKernelResults

# wall layout (single [128, WALL_W] f32 constant DMA)
WG_OFF = 0                 # 8 x [128,128]: (Wf a|b, Wi a|b, Wu a|b, Wo a|b)^T
WEL_OFF = 1024             # [64, 128] W_el^T on partitions 0:64
WA_OFF = 1152              # [64, 4]  [W_eoh | b_eoh] on partitions 0:64
BEL_OFF = 1156             # [4, 128] row 3 = b_el, on partitions 0:4
BIAS_OFF = 1284            # 8 x [128,1]: bWf, bf, bWi, bi, bWu, bu, bWo, bo
WALL_W = 1292


def _install_axon_hook():
    import types, contextlib, ctypes

    def _make_hook(so_path="/opt/axon/libaxon_pjrt.so"):
        lib = ctypes.CDLL(so_path)
        if not hasattr(lib, "axon_start_nrt_profile"):
            return None
        lib.axon_start_nrt_profile.argtypes = [
            ctypes.POINTER(ctypes.c_int64), ctypes.c_size_t]
        lib.axon_start_nrt_profile.restype = ctypes.c_int64
        lib.axon_stop_nrt_profile.argtypes = [ctypes.c_char_p]
        lib.axon_stop_nrt_profile.restype = ctypes.c_int64

        @contextlib.contextmanager
        def hook(output_dir, device_ids):
            import jax
            jax.devices()
            if device_ids:
                ids = (ctypes.c_int64 * len(device_ids))(*device_ids)
                rc = lib.axon_start_nrt_profile(ids, len(device_ids))
            else:
                rc = lib.axon_start_nrt_profile(None, 0)
            if rc != 0:
                raise RuntimeError("axon_start_nrt_profile rc=%d" % rc)
            try:
                yield
            finally:
                n = lib.axon_stop_nrt_profile(str(output_dir).encode())
                print("profile: %d file(s) written to %s" % (n, output_dir),
                      file=sys.stderr)

        return hook

    hook = _make_hook()
    mod = types.ModuleType("antenv.axon_hooks")
    mod.get_axon_ntff_profile_hook = lambda: hook
    mod.set_axon_ntff_profile_hook = lambda h: None
    sys.modules["antenv.axon_hooks"] = mod


def build_graph(SB):
    """Per-core Bass graph for SB superblocks."""
    nc = bacc.Bacc()
    assert SB % 2 == 0 and SB >= 6
    dp = nc.declare_dram_parameter
    D8W = 2 * BLK_E            # per-sb cols in d8: h 2048 | e 2048
    d8_ext = dp("d8", [SB // 2, 128, 2 * D8W], F8E3, isOutput=False)
    dm4_ext = dp("dm4", [SB // 2, 128, 2 * BLK_N * SUB_C], F8E4, isOutput=False)
    d16_ext = dp("d16", [SB // 2, 128, 2 * BLK_E], BF16, isOutput=False)
    SFW = 2 * BLK_E            # per-sb cols in sfdf: sf 2048 | df 2048
    sfdf_ext = dp("sfdf", [SB // 2, G + 4, 2 * SFW], F8E3, isOutput=False)
    wall_ext = dp("wall", [128, WALL_W], F32, isOutput=False)
    out_ext = dp("outT", [128, SB * 2 * BLK_N], BF16, isOutput=True)

    AF = mybir.ActivationFunctionType
    PM = mybir.MatmulPerfMode
    M4W = BLK_N * SUB_C        # per-sb cols in dm4 (1024)

    with tile.TileContext(nc) as tc:
        cst = tc.alloc_tile_pool(name="cst", bufs=1)
        pin = tc.alloc_tile_pool(name="pin", bufs=5)
        pcv = tc.alloc_tile_pool(name="pcv", bufs=4)
        pnd = tc.alloc_tile_pool(name="pnd", bufs=3)
        pew = tc.alloc_tile_pool(name="pew", bufs=2, space="PSUM")
        phs = tc.alloc_tile_pool(name="phs", bufs=1, space="PSUM")
        pcs = tc.alloc_tile_pool(name="pcs", bufs=2, space="PSUM")
        pgp = tc.alloc_tile_pool(name="pgp", bufs=2, space="PSUM")

        # -- constants first: small DMA on the scalar hwdge queue, which
        # carries no bulk traffic, so wtcomb assembly finishes ~4us in ------
        wall = cst.tile([128, WALL_W], F32)
        nc.scalar.dma_start(out=wall[:], in_=wall_ext[:])

        # Input fetches split over two DMA queue groups: d8+m4 on the sync
        # (SP hwdge) queue, sfdf+d16 on the gpsimd queue.  The scalar queue
        # only carries constants — in-loop triggers on a busy engine queue
        # block it when they wait for a free tile buffer.  sfdf first
        # within the pair (unblocks B68 + ew).
        def fetch_pair(p):
            sfdf = pin.tile([G + 4, 2 * SFW], F8E3, tag="sfdf")
            nc.gpsimd.dma_start(out=sfdf[:], in_=sfdf_ext[p])
            d8 = pin.tile([128, 2 * D8W], F8E3, tag="d8")
            nc.sync.dma_start(out=d8[:], in_=d8_ext[p])
            m4 = pin.tile([128, 2 * M4W], F8E4, tag="m4")
            nc.sync.dma_start(out=m4[:], in_=dm4_ext[p])
            d16 = pin.tile([128, 2 * BLK_E], BF16, tag="d16")
            nc.gpsimd.dma_start(out=d16[:], in_=d16_ext[p])
            return (d8, d16, m4, sfdf)

        pairs = {0: fetch_pair(0), 1: fetch_pair(1), 2: fetch_pair(2)}

        t2p = pgp.tile([4, 128], F32, tag="mm")
        nc.tensor.matmul(out=t2p[:], lhsT=wall[0:G, WA_OFF:WA_OFF + 4],
                         rhs=wall[0:G, WEL_OFF:WEL_OFF + 128],
                         start=True, stop=True)
        t4b = cst.tile([4, 128], BF16)
        nc.vector.tensor_tensor(out=t4b[:], in0=t2p[:],
                                in1=wall[0:4, BEL_OFF:BEL_OFF + 128],
                                op=mybir.AluOpType.add)
        wel_b16 = cst.tile([G, 128], BF16)
        nc.vector.tensor_copy(out=wel_b16[:],
                              in_=wall[0:G, WEL_OFF:WEL_OFF + 128])
        wtcomb = cst.tile([G + 4, 128], BF16)
        nc.scalar.dma_start(out=wtcomb[0:G, :], in_=wel_b16[:])
        nc.scalar.dma_start(out=wtcomb[G:G + 4, :], in_=t4b[:])

        wg = {}
        for xi, x in enumerate("fiuo"):
            wa_t = cst.tile([128, 128], BF16, tag="wg_%s_a" % x)
            nc.vector.tensor_copy(
                out=wa_t[:], in_=wall[:, 256 * xi:256 * xi + 128])
            wb_t = cst.tile([128, 128], BF16, tag="wg_%s_b" % x)
            nc.gpsimd.tensor_copy(
                out=wb_t[:], in_=wall[:, 256 * xi + 128:256 * xi + 256])
            wg[x] = (wa_t, wb_t)

        bias = {}
        for xi, x in enumerate("fiuo"):
            bs = cst.tile([128, 1], F32, tag="bs_%s" % x)
            nc.vector.tensor_tensor(
                out=bs[:], in0=wall[:, BIAS_OFF + 2 * xi:BIAS_OFF + 2 * xi + 1],
                in1=wall[:, BIAS_OFF + 2 * xi + 1:BIAS_OFF + 2 * xi + 2],
                op=mybir.AluOpType.add)
            bias[x] = bs

        # -- per-superblock emission helpers --------------------------------
        # B68 = sf*df products; Pool is ~2.1x slower per column than DVE so
        # the split is asymmetric, and the two halves are emitted at
        # different points (Pool early, DVE late) to order each queue well.
        BQ = 704                   # B68 columns computed on DVE

        def emit_b68_pool(g, sfdf):
            j = (g % 2) * SFW
            B68 = pcv.tile([G + 4, BLK_E], BF16, tag="B68")
            nc.gpsimd.tensor_tensor(
                out=B68[:, BQ:BLK_E],
                in0=sfdf[:, j + BQ:j + BLK_E],
                in1=sfdf[:, j + BLK_E + BQ:j + 2 * BLK_E],
                op=mybir.AluOpType.mult)
            return B68

        def emit_b68_dve(g, B68, sfdf):
            j = (g % 2) * SFW
            nc.vector.tensor_tensor(
                out=B68[:, 0:BQ], in0=sfdf[:, j:j + BQ],
                in1=sfdf[:, j + BLK_E:j + BLK_E + BQ],
                op=mybir.AluOpType.mult)

        def emit_gates(g, hsab):
            gate = {}
            for x, fn in (("f", "Sigmoid"), ("i", "Sigmoid"),
                          ("u", "Tanh"), ("o", "Sigmoid")):
                gp = pgp.tile([128, BLK_N], F32, tag="mm")
                nc.tensor.matmul(out=gp[:], lhsT=wg[x][0][:],
                                 rhs=hsab[:, 0:BLK_N], start=True, stop=False)
                nc.tensor.matmul(out=gp[:], lhsT=wg[x][1][:],
                                 rhs=hsab[:, BLK_N:2 * BLK_N],
                                 start=False, stop=True)
                gs = pnd.tile([128, BLK_N], BF16, tag="g_%s" % x)
                nc.scalar.activation(out=gs[:], in_=gp[:],
                                     func=getattr(AF, fn), bias=bias[x][:])
                gate[x] = gs
            return gate

        hc2_ref = [None]

        def emit_assembly_head(g, gate, cs_ps):
            # node-level, all on DVE (bf16 2x) + ACT tanh: ct = f*cs (cs read
            # straight from PSUM), c = i*u + ct, th = tanh(c)
            ct = pnd.tile([128, BLK_N], BF16, tag="ct")
            nc.vector.tensor_tensor(out=ct[:], in0=gate["f"][:],
                                    in1=cs_ps[:], op=mybir.AluOpType.mult)
            iu = pnd.tile([128, BLK_N], BF16, tag="iu")
            nc.vector.tensor_tensor(out=iu[:], in0=gate["i"][:],
                                    in1=gate["u"][:], op=mybir.AluOpType.mult)
            if g % 2 == 0:
                hc2_ref[0] = pnd.tile([128, 4 * BLK_N], BF16, tag="hc",
                                      name="hc2")
            hc = hc2_ref[0]
            o2 = (g % 2) * 2 * BLK_N
            nc.vector.tensor_tensor(out=hc[:, o2 + BLK_N:o2 + 2 * BLK_N],
                                    in0=iu[:], in1=ct[:],
                                    op=mybir.AluOpType.add)
            th = pnd.tile([128, BLK_N], BF16, tag="th")
            nc.scalar.activation(out=th[:], in_=hc[:, o2 + BLK_N:
                                                  o2 + 2 * BLK_N],
                                 func=AF.Tanh)
            return (g, gate, hc, th)

        def emit_assembly_tail(g, gate, hc, th):
            # h = o*tanh(c) on Pool (it idles at this point of the
            # pipeline; on DVE this op measured 3-5x slow — it runs right
            # when the DVE would rather stream the next hw4).  The output
            # DMA trigger rides the SAME engine queue directly behind hmul
            # so it never blocks the queue waiting on hc; output ships in
            # 2-superblock batches.
            o2 = (g % 2) * 2 * BLK_N
            nc.gpsimd.tensor_tensor(out=hc[:, o2:o2 + BLK_N],
                                    in0=gate["o"][:],
                                    in1=th[:], op=mybir.AluOpType.mult)
            if g % 2 == 1 or g == SB - 1:
                g0 = g - (g % 2)
                nc.gpsimd.dma_start(
                    out=out_ext[:, g0 * 2 * BLK_N:(g0 + 2) * 2 * BLK_N],
                    in_=hc[:])

        # -- main loop -------------------------------------------------------
        # Software pipeline: B68 products run TWO superblocks ahead and the
        # whole ew->hw4 stream runs ONE superblock ahead, so PE's h-segsum
        # never waits on same-iteration DVE work.
        def d8h_of(j):
            d8p = pairs[j // 2][0]
            return d8p[:, (j % 2) * D8W:(j % 2) * D8W + BLK_E]

        def ew_quarter(j, q, B68j, hw4j):
            # 4 ew matmuls -> one hw4 = h*ew quarter on DVE (superblock j)
            ew_ps = pew.tile([128, 512], F32, tag="ew")
            for c in range(4):
                ch = q * 4 + c
                nc.tensor.matmul(
                    out=ew_ps[:, c * 128:(c + 1) * 128],
                    lhsT=B68j[:, ch * 128:(ch + 1) * 128],
                    rhs=wtcomb[:], start=True, stop=True)
            nc.vector.tensor_tensor(
                out=hw4j[:, q * 512:(q + 1) * 512],
                in0=d8h_of(j)[:, q * 512:(q + 1) * 512], in1=ew_ps[:],
                op=mybir.AluOpType.mult)

        B68s = {0: emit_b68_pool(0, pairs[0][3])}
        emit_b68_dve(0, B68s[0], pairs[0][3])
        B68s[1] = emit_b68_pool(1, pairs[0][3])
        emit_b68_dve(1, B68s[1], pairs[0][3])
        hw4s = {0: pcv.tile([128, BLK_E], F8E4, tag="hw4", name="hw40")}
        for q in range(4):
            ew_quarter(0, q, B68s[0], hw4s[0])

        prev = None     # (g, gate, cs_ps) awaiting assembly
        evac = None     # (g, hsx, cs_ps) awaiting gates
        for g in range(SB):
            if g % 2 == 1 and (g + 5) // 2 < SB // 2:
                pairs[(g + 5) // 2] = fetch_pair((g + 5) // 2)
            d8p, d16p, m4p, _ = pairs[g // 2]
            j8 = (g % 2) * D8W
            jm = (g % 2) * M4W
            j16 = (g % 2) * BLK_E
            d8e = d8p[:, j8 + BLK_E:j8 + D8W]
            m4 = m4p[:, jm:jm + M4W]
            d16c = d16p[:, j16:j16 + BLK_E]

            if g + 2 < SB:
                B68s[g + 2] = emit_b68_pool(g + 2, pairs[(g + 2) // 2][3])
            B68nx = B68s.get(g + 1)
            hw4 = hw4s[g]
            if g + 1 < SB:
                hw4s[g + 1] = pcv.tile([128, BLK_E], F8E4, tag="hw4",
                                       name="hw4n")

            # node-level head of superblock g-1 (ct/iu/add lead the DVE
            # queue so the c-segsum + pnd consumers unblock early)
            asm = None
            if prev is not None:
                asm = emit_assembly_head(prev[0], prev[1], prev[2])
                prev = None

            hs = phs.tile([128, 2 * BLK_N], F32, tag="hs")
            cs = pcs.tile([128, BLK_N], F32, tag="cs")

            def cseg(st_lo, st_hi):
                for st in range(st_lo, st_hi):
                    for k in range(SUB_C):
                        ch = st * SUB_C + k
                        nc.tensor.matmul(
                            out=cs[:, st * SUB_N:(st + 1) * SUB_N],
                            lhsT=d16c[:, ch * 128:(ch + 1) * 128],
                            rhs=m4[:, ch * SUB_N:(ch + 1) * SUB_N],
                            start=(k == 0), stop=(k == SUB_C - 1))

            def eseg(st_lo, st_hi):
                for st in range(st_lo, st_hi):
                    for k in range(SUB_C):
                        ch = st * SUB_C + k
                        nc.tensor.matmul(
                            out=hs[:, BLK_N + st * SUB_N:
                                   BLK_N + (st + 1) * SUB_N],
                            lhsT=d8e[:, ch * 128:(ch + 1) * 128],
                            rhs=m4[:, ch * SUB_N:(ch + 1) * SUB_N],
                            start=(k == 0), stop=(k == SUB_C - 1))

            def hseg(st_lo, st_hi):
                # h-part segment sum: one DoubleRow fp8 matmul per subtile;
                # hw4 of THIS superblock was produced last iteration
                for st in range(st_lo, st_hi):
                    nc.tensor.matmul(
                        out=hs[:, st * SUB_N:(st + 1) * SUB_N],
                        lhsT=hw4[:, st * SUB_E:(st + 1) * SUB_E].rearrange(
                            "p (k m) -> p k m", k=2),
                        rhs=m4[:, st * SUB_C * SUB_N:(st + 1) * SUB_C * SUB_N]
                            .rearrange("p (k n) -> p k n", k=2),
                        start=True, stop=True, perf_mode=PM.DoubleRow)

            def phase(q):
                if g + 1 < SB:
                    ew_quarter(g + 1, q, B68nx, hw4s[g + 1])
                cseg(2 * q, 2 * q + 2)
                eseg(2 * q, 2 * q + 2)
                hseg(2 * q, 2 * q + 2)

            phase(0)
            phase(1)
            phase(2)

            # h = o*tanh(c) of g-1 (Pool) + its output DMA
            if asm is not None:
                emit_assembly_tail(*asm)
                asm = None

            # gates of the previous superblock between phases so the ACT
            # sigmoids land early while PE still has phase-3 work queued
            if evac is not None:
                prev = (evac[0], emit_gates(evac[0], evac[1]), evac[2])

            phase(3)

            # evacuate hs (ACT) behind the gate sigmoids
            hsx = pnd.tile([128, 2 * BLK_N], BF16, tag="hsx")
            nc.scalar.activation(out=hsx[:], in_=hs[:], func=AF.Copy)

            evac = (g, hsx, cs)
            if g + 2 < SB:
                emit_b68_dve(g + 2, B68s[g + 2], pairs[(g + 2) // 2][3])
            hw4s.pop(g, None)
            B68s.pop(g, None)

        if prev is not None:
            asm = emit_assembly_head(prev[0], prev[1], prev[2])
            emit_assembly_tail(*asm)
        prev = (evac[0], emit_gates(evac[0], evac[1]), evac[2])
        asm = emit_assembly_head(prev[0], prev[1], prev[2])
        emit_assembly_tail(*asm)

        for p in (pgp, pcs, phs, pew, pnd, pcv, pin, cst):
            p.release()
    nc.finalize()
    return nc


def plan_subtiles(dst_local, npc):
    """Greedy: <=SUB_N nodes and <=SUB_E edges per subtile.
    Returns list of (n0, n1, e0, e1) using sorted-edge offsets."""
    cnt = np.bincount(dst_local, minlength=npc)
    cum = np.concatenate([[0], np.cumsum(cnt)])
    tiles = []
    s = 0
    while s < npc:
        hi = min(s + SUB_N, npc)
        m = int(np.searchsorted(cum, cum[s] + SUB_E, side="right")) - 1
        m = max(s + 1, min(hi, m))
        tiles.append((s, m, int(cum[s]), int(cum[m])))
        s = m
    return tiles


def prep_core(k, h_src, c_src, embed_dst, src_f, dst_f, etype, dst, SB):
    """Build one core's padded superblock arrays."""
    lo = k * NPC
    sel = np.nonzero((dst >= lo) & (dst < lo + NPC))[0]
    dl = (dst[sel] - lo).astype(np.int64)
    order = np.argsort(dl, kind="stable")
    eidx = sel[order]
    dls = dl[order]
    tiles = plan_subtiles(dls, NPC)
    T = SB * SPB
    assert len(tiles) <= T
    ES = T * SUB_E
    src_slot = np.full(ES, -1, dtype=np.int64)
    nl_slot = np.zeros(ES, dtype=np.int64)      # node idx within subtile
    for t, (n0, n1, e0, e1) in enumerate(tiles):
        ne = e1 - e0
        assert ne <= SUB_E and n1 - n0 <= SUB_N
        src_slot[t * SUB_E:t * SUB_E + ne] = eidx[e0:e1]
        nl_slot[t * SUB_E:t * SUB_E + ne] = dls[e0:e1] - n0
    val = src_slot >= 0
    gi = src_slot[val]

    def pad_rows(a, w):
        out = np.zeros((ES, w), dtype=np.float32)
        out[val] = a[gi]
        return out

    def chunk_layout(a, w):
        # [ES, w] -> [SB, 128, BLK_C*w]: slot (sb, ch, p) dim d at
        # [sb, p, ch*w + d]
        return np.ascontiguousarray(
            a.reshape(SB, BLK_C, CHUNK, w).transpose(0, 2, 1, 3)
             .reshape(SB, 128, BLK_C * w))

    # membership: [sb, p, ch*64 + j] = (nl_slot of (sb,ch,p) == j)
    nl = nl_slot.reshape(SB, BLK_C, CHUNK)
    vl = val.reshape(SB, BLK_C, CHUNK)
    m4 = (nl[:, :, :, None] == np.arange(SUB_N)[None, None, None, :])
    m4 = (m4 & vl[:, :, :, None]).astype(np.float32)
    m4 = m4.reshape(SB, BLK_C, CHUNK, SUB_N).transpose(0, 2, 1, 3) \
           .reshape(SB, 128, BLK_C * SUB_N)

    def pair(a):
        # [SB, P, W] -> [SB/2, P, 2W]
        S, P, W = a.shape
        return np.ascontiguousarray(
            a.reshape(S // 2, 2, P, W).transpose(0, 2, 1, 3)
             .reshape(S // 2, P, 2 * W))

    h8 = chunk_layout(pad_rows(h_src, H), H)
    e8 = chunk_layout(pad_rows(embed_dst, H), H)
    d8 = pair(np.concatenate([h8, e8], axis=2)).astype(f8e3_np)
    dm4 = pair(m4).astype(f8e4_np)

    cp = chunk_layout(pad_rows(c_src, H), H)
    d16 = pair(cp).astype(bf16_np)

    # sf' = [sf | onehot4], df' = [df | ones]: [SB, 68, ch*128 + p]
    sfp = np.zeros((ES, G + 4), dtype=np.float32)
    sfp[val, :G] = src_f[gi]
    sfp[val, G + etype[gi]] = 1.0
    sfp[val, G + 3] = 1.0
    dfp = np.zeros((ES, G + 4), dtype=np.float32)
    dfp[val, :G] = dst_f[gi]
    dfp[val, G:] = 1.0
    def feat_layout(a):
        return a.reshape(SB, BLK_C * CHUNK, G + 4).transpose(0, 2, 1)
    sfdf = pair(np.concatenate(
        [feat_layout(sfp), feat_layout(dfp)], axis=2)).astype(f8e3_np)

    return {"d8": d8, "dm4": dm4, "d16": d16, "sfdf": sfdf}, tiles


def build_wall(inputs):
    wall = np.zeros((128, WALL_W), dtype=np.float32)
    for xi, (wn, bwn, bn) in enumerate(
            (("Wf", "bWf", "bf"), ("Wi", "bWi", "bi"),
             ("Wu", "bWu", "bu"), ("Wo", "bWo", "bo"))):
        wT = np.asarray(inputs[wn], np.float32).T         # [256, 128]
        wall[:, 256 * xi:256 * xi + 128] = wT[0:128]
        wall[:, 256 * xi + 128:256 * xi + 256] = wT[128:256]
        wall[:, BIAS_OFF + 2 * xi] = np.asarray(inputs[bwn], np.float32)
        wall[:, BIAS_OFF + 2 * xi + 1] = np.asarray(inputs[bn], np.float32)
    wall[0:G, WEL_OFF:WEL_OFF + 128] = np.asarray(inputs["W_el"], np.float32).T
    wall[0:G, WA_OFF:WA_OFF + 3] = np.asarray(inputs["W_eoh"], np.float32)
    wall[0:G, WA_OFF + 3] = np.asarray(inputs["b_eoh"], np.float32)
    wall[3, BEL_OFF:BEL_OFF + 128] = np.asarray(inputs["b_el"], np.float32)
    return wall


_graph_cache = {}


def kernel(**inputs):
    h_src = np.asarray(inputs["h_src"], dtype=np.float32)
    c_src = np.asarray(inputs["c_src"], dtype=np.float32)
    embed_dst = np.asarray(inputs["embed_dst"], dtype=np.float32)
    src_f = np.asarray(inputs["src_node_feat"], dtype=np.float32)
    dst_f = np.asarray(inputs["dst_node_feat"], dtype=np.float32)
    etype = np.asarray(inputs["edge_type_idx"]).astype(np.int64)
    dst = np.asarray(inputs["dst_idx"]).astype(np.int64)

    wall = build_wall(inputs)

    planned = []
    for k in range(NCORES):
        lo = k * NPC
        sel = np.nonzero((dst >= lo) & (dst < lo + NPC))[0]
        dl = np.sort((dst[sel] - lo).astype(np.int64))
        planned.append(plan_subtiles(dl, NPC))
    T = max(len(p) for p in planned)
    SB = (T + SPB - 1) // SPB
    SB += SB % 2
    SB = max(SB, 6)

    in_maps = []
    tiles_all = []
    for k in range(NCORES):
        m, tiles = prep_core(k, h_src, c_src, embed_dst, src_f, dst_f,
                             etype, dst, SB)
        m["wall"] = wall
        in_maps.append(m)
        tiles_all.append(tiles)

    if SB not in _graph_cache:
        _graph_cache[SB] = build_graph(SB)
    nc = _graph_cache[SB]

    if TRACE:
        _install_axon_hook()
    res = run_bass_kernel_spmd(nc, in_maps, list(range(NCORES)), trace=TRACE)
    LAST["res"] = res

    out = np.empty((N, 2 * H), dtype=np.float32)
    for k in range(NCORES):
        outT = np.asarray(res.results[k]["outT"]).astype(np.float32)
        for t, (n0, n1, _, _) in enumerate(tiles_all[k]):
            nn = n1 - n0
            base = k * NPC
            sb, st = divmod(t, SPB)
            col = sb * 2 * BLK_N + st * SUB_N
            out[base + n0:base + n1, 0:H] = outT[:, col:col + nn].T
            out[base + n0:base + n1, H:2 * H] = \
                outT[:, col + BLK_N:col + BLK_N + nn].T
    return out
